# revision 9
# baseline (speedup 1.0000x reference)
"""GCN (2-layer + linear head) Trainium2 kernel, 8-core SPMD, 3 NEFF launches.

Math (reference-equivalent, using linearity of segment_sum and b1=b2=0):
    A = D^-1/2 (Adj+I) D^-1/2
    y = relu(A relu(A x W1) W2) Wout + bout
computed as
    M1 = diag(dinv) x                        (bf16 gather table)   [NEFF A]
    agg1T_raw[:, d] = sum_{e: dst=d} M1[src_e]                     [NEFF B]
    h1T = relu(W1^T agg1T_raw)      (dst dinv commutes thru relu, b1=0)
    M2 = diag(dinv^2) (h1 W2)       (bf16 gather table)
    agg2T_raw[:, d] = sum_{e: dst=d} M2[src_e]                     [NEFF C]
    y = diag(dinv) (relu(agg2T_raw)^T Wout) + bout

Between launches the host only concatenates the 8 per-core shard outputs
into the full gather table (no arithmetic on host beyond graph indexing).

Per core: 12500 dst nodes, ~212k edges (incl self loops), edges sorted by
(superblock of 4 dst tiles, src bucket of 25000, dst tile). Each 128-edge
chunk is single (bucket, tile): dma_gather (int16 idx relative to bucket
base) fetches the 128 source rows (partition=edge), a one-hot S matrix
(iota is_equal dstloc, bf16) scatters them via PE matmul into the tile's
PSUM accumulator in transposed orientation out[feat, dst] — so the dense
per-tile matmul chain needs no transposes anywhere.
"""

import os
import sys

if "/opt/trn_rl_repo" not in sys.path:
    sys.path.insert(0, "/opt/trn_rl_repo")

import numpy as np
import ml_dtypes

import concourse.bacc as bacc
import concourse.mybir as mybir
import concourse.tile as tile
from concourse import library_config
from concourse.bass_utils import run_bass_kernel_spmd

P = 128
N_CORES = 8
IN_DIM = 128
HID_DIM = 256
OUT_DIM = 128
BUCKET = 25000                    # gather-window size; int16 idx < 32768
SBT = 4                           # dst tiles per superblock (psum residency)
MAXG = 8                          # chunks per dma_gather (1024 idx ring cap)
BF16 = mybir.dt.bfloat16
F32 = mybir.dt.float32


def _register_ntff_hook():
    """Make trace=True usable under axon when antenv.axon_hooks is absent."""
    try:
        import antenv.axon_hooks  # noqa: F401
        return
    except ImportError:
        pass
    import types
    import antenv
    mod = types.ModuleType("antenv.axon_hooks")
    _h = [None]
    mod.set_axon_ntff_profile_hook = lambda h: _h.__setitem__(0, h)
    mod.get_axon_ntff_profile_hook = lambda: _h[0]
    sys.modules["antenv.axon_hooks"] = mod
    antenv.axon_hooks = mod
    try:
        from trn_agent_boot.trn_boot import _ntff_profile_via_ctypes
        mod.set_axon_ntff_profile_hook(
            _ntff_profile_via_ctypes("/opt/axon/libaxon_pjrt.so"))
    except Exception:
        pass


# --------------------------------------------------------------------------
# host-side graph preprocessing (pure index space)
# --------------------------------------------------------------------------

def _preprocess_core(s, d, n_tiles, n_buckets):
    """Edges (global src s, core-local dst d) -> padded chunk stream."""
    tile_id = d >> 7
    bucket = s // BUCKET
    sb = tile_id // SBT
    key = (sb.astype(np.int64) * n_buckets + bucket) * n_tiles + tile_id
    order = np.argsort(key, kind="stable")
    s, d, tile_id, bucket, key = (a[order] for a in (s, d, tile_id, bucket, key))

    bounds = np.flatnonzero(np.diff(key)) + 1
    starts = np.concatenate([[0], bounds])
    ends = np.concatenate([bounds, [len(s)]])

    idx_parts, dl_parts = [], []
    chunk_tile, chunk_bucket = [], []
    for lo, hi in zip(starts, ends):
        n = hi - lo
        npad = -n % P
        t = int(tile_id[lo])
        b = int(bucket[lo])
        idx = (s[lo:hi] - b * BUCKET).astype(np.int16)
        dl = (d[lo:hi] - t * P).astype(np.float32)
        if npad:
            idx = np.concatenate([idx, np.zeros(npad, np.int16)])
            dl = np.concatenate([dl, np.full(npad, -1.0, np.float32)])
        idx_parts.append(idx)
        dl_parts.append(dl)
        nch = (n + npad) // P
        chunk_tile += [t] * nch
        chunk_bucket += [b] * nch

    return dict(idx=np.concatenate(idx_parts),
                dstloc=np.concatenate(dl_parts),
                chunk_tile=np.asarray(chunk_tile),
                chunk_bucket=np.asarray(chunk_bucket))


def _merge_meta(core_meta, n_tiles, n_buckets):
    """Unify per-core chunk structure so one NEFF serves all 8 cores.

    Canonical chunk count per (tile, bucket) = max across cores; pad chunks
    (idx=0, dstloc=-1) contribute nothing."""
    counts = np.zeros((N_CORES, n_tiles, n_buckets), np.int64)
    for c, m in enumerate(core_meta):
        np.add.at(counts[c], (m["chunk_tile"], m["chunk_bucket"]), 1)
    maxc = counts.max(axis=0)

    chunk_tile, chunk_bucket = [], []
    n_sb = -(-n_tiles // SBT)
    for sbi in range(n_sb):
        tiles = range(sbi * SBT, min((sbi + 1) * SBT, n_tiles))
        for b in range(n_buckets):
            for t in tiles:
                n = int(maxc[t, b])
                chunk_tile += [t] * n
                chunk_bucket += [b] * n
    chunk_tile = np.asarray(chunk_tile)
    chunk_bucket = np.asarray(chunk_bucket)
    nch = len(chunk_tile)

    first = np.zeros(nch, bool)
    last = np.zeros(nch, bool)
    seen = {}
    for i, t in enumerate(chunk_tile):
        if t not in seen:
            first[i] = True
        seen[int(t)] = i
    for t, i in seen.items():
        last[i] = True

    groups = []
    i = 0
    while i < nch:
        b = chunk_bucket[i]
        sb_i = chunk_tile[i] // SBT
        j = i
        while (j < nch and chunk_bucket[j] == b
               and chunk_tile[j] // SBT == sb_i and j - i < MAXG):
            j += 1
        groups.append((int(b), i, j))
        i = j

    # canonical slots per (tile,bucket) in order
    slot = {}
    for i in range(nch):
        slot.setdefault((int(chunk_tile[i]), int(chunk_bucket[i])), []).append(i)

    core_idx, core_dstloc = [], []
    for m in core_meta:
        idx_new = np.zeros((nch, P), np.int16)
        dl_new = np.full((nch, P), -1.0, np.float32)
        used = {}
        own_idx = m["idx"].reshape(-1, P)
        own_dl = m["dstloc"].reshape(-1, P)
        for ci in range(len(m["chunk_tile"])):
            key = (int(m["chunk_tile"][ci]), int(m["chunk_bucket"][ci]))
            k = used.get(key, 0)
            used[key] = k + 1
            dst_ci = slot[key][k]
            idx_new[dst_ci] = own_idx[ci]
            dl_new[dst_ci] = own_dl[ci]
        # device layouts: idx [128, nch*8] (16-wrap replicated x8),
        # dstloc [128, nch]
        idx16 = idx_new.reshape(-1, 16).T.copy()      # [16, nch*8]
        core_idx.append(np.tile(idx16, (8, 1)))       # [128, nch*8]
        core_dstloc.append(dl_new.T.copy())           # [128, nch]

    return dict(chunk_tile=chunk_tile, first=first, last=last, groups=groups,
                n_chunks=nch, core_idx=core_idx, core_dstloc=core_dstloc)


# --------------------------------------------------------------------------
# NEFF builders
# --------------------------------------------------------------------------

def _new_nc(sim_mode):
    return bacc.Bacc("TRN2", target_bir_lowering=False, debug=sim_mode,
                     num_devices=N_CORES)


def _build_prescale(nc, n_tiles, n_valid):
    """NEFF A: m1s = bf16(diag(dinv) x_shard)."""
    t_x = nc.dram_tensor("xs", [n_tiles * P, IN_DIM], F32, kind="ExternalInput")
    t_di = nc.dram_tensor("dinv_pp", [P, n_tiles], F32, kind="ExternalInput")
    t_m1 = nc.dram_tensor("m1s", [n_valid, IN_DIM], BF16, kind="ExternalOutput")
    with tile.TileContext(nc) as tc:
        with tc.tile_pool(name="w", bufs=4) as wpool, \
             tc.tile_pool(name="c", bufs=1) as cpool:
            di_sb = cpool.tile([P, n_tiles], F32)
            nc.sync.dma_start(di_sb[:], t_di[:])
            for t in range(n_tiles):
                v = min(P, n_valid - t * P)
                x_sb = wpool.tile([P, IN_DIM], F32, tag="x")
                nc.sync.dma_start(x_sb[:], t_x[t * P:(t + 1) * P, :])
                m1_sb = wpool.tile([P, IN_DIM], BF16, tag="m1")
                nc.vector.tensor_scalar_mul(m1_sb[:], x_sb[:], di_sb[:, t:t + 1])
                nc.sync.dma_start(t_m1[t * P:t * P + v, :], m1_sb[:v, :])


def _emit_aggregation(nc, meta, pools, table_ap, idx_sb, dl_sb, iota_sb,
                      feat_dim, n_nodes, tail_fn):
    """Gather + one-hot scatter matmuls; tail_fn(t, agg_psum) per dst tile."""
    mpool, spool, agg_pp = pools
    psums = {}
    for (b, c_lo, c_hi) in meta["groups"]:
        nch = c_hi - c_lo
        nidx = nch * P
        msg = mpool.tile([P, MAXG, feat_dim], BF16, tag="msg")
        nc.gpsimd.dma_gather(
            msg[:, :nch, :],
            table_ap[b * BUCKET:min((b + 1) * BUCKET, n_nodes), :],
            idx_sb[:, c_lo * 8:c_hi * 8],
            nidx, nidx, feat_dim)
        for k in range(nch):
            ci = c_lo + k
            t = int(meta["chunk_tile"][ci])
            if meta["first"][ci]:
                psums[t] = agg_pp.tile([P, P], F32, tag="agg",
                                       name=f"aggps_{t}")
            S = spool.tile([P, P], BF16, tag="S")
            nc.vector.tensor_scalar(S[:], iota_sb[:], dl_sb[:, ci:ci + 1],
                                    None, op0=mybir.AluOpType.is_equal)
            nc.tensor.matmul(psums[t][:], lhsT=msg[:, k, :], rhs=S[:],
                             start=bool(meta["first"][ci]),
                             stop=bool(meta["last"][ci]))
            if meta["last"][ci]:
                tail_fn(t, psums.pop(t))


def _build_layer1(nc, meta, n_tiles, n_valid, n_nodes):
    """NEFF B: gather M1, aggregate, h1T = relu(W1^T aggT),
    m2s = bf16(diag(dinv^2) (h1 W2))."""
    NC_ = meta["n_chunks"]
    t_tbl = nc.dram_tensor("m1full", [n_nodes, IN_DIM], BF16,
                           kind="ExternalInput")
    t_idx = nc.dram_tensor("idx", [P, NC_ * 8], mybir.dt.int16,
                           kind="ExternalInput")
    t_dl = nc.dram_tensor("dstloc", [P, NC_], F32, kind="ExternalInput")
    t_iota = nc.dram_tensor("iota", [P, P], BF16, kind="ExternalInput")
    t_w1 = nc.dram_tensor("w1", [IN_DIM, HID_DIM], F32, kind="ExternalInput")
    t_w2 = nc.dram_tensor("w2p", [P, HID_DIM], F32, kind="ExternalInput")
    t_di2 = nc.dram_tensor("dinv2_pp", [P, n_tiles], F32, kind="ExternalInput")
    t_m2 = nc.dram_tensor("m2s", [n_valid, OUT_DIM], BF16,
                          kind="ExternalOutput")
    with tile.TileContext(nc) as tc:
        with (
            tc.tile_pool(name="const", bufs=1) as cpool,
            tc.tile_pool(name="msg", bufs=4) as mpool,
            tc.tile_pool(name="s", bufs=4) as spool,
            tc.tile_pool(name="work", bufs=3) as wpool,
            tc.tile_pool(name="agg_ps", bufs=4, space="PSUM") as agg_pp,
            tc.tile_pool(name="h1_ps", bufs=2, space="PSUM") as h1_pp,
            tc.tile_pool(name="m2_ps", bufs=2, space="PSUM") as m2_pp,
        ):
            nc.gpsimd.load_library(library_config.mlp)
            idx_sb = cpool.tile([P, NC_ * 8], mybir.dt.int16)
            dl_sb = cpool.tile([P, NC_], F32)
            iota_sb = cpool.tile([P, P], BF16)
            w1_sb = cpool.tile([IN_DIM, HID_DIM], F32)
            w2_sb = cpool.tile([P, HID_DIM], F32)
            di2_sb = cpool.tile([P, n_tiles], F32)
            nc.sync.dma_start(idx_sb[:], t_idx[:])
            nc.sync.dma_start(dl_sb[:], t_dl[:])
            nc.sync.dma_start(iota_sb[:], t_iota[:])
            nc.sync.dma_start(w1_sb[:], t_w1[:])
            nc.sync.dma_start(w2_sb[:], t_w2[:])
            nc.sync.dma_start(di2_sb[:], t_di2[:])

            def l1_tail(t, agg_ps):
                v = min(P, n_valid - t * P)
                aggT = wpool.tile([P, P], F32, tag="aggT")
                nc.vector.tensor_copy(aggT[:], agg_ps[:])
                h1_ps = h1_pp.tile([P, HID_DIM], F32, tag="h1")
                nc.tensor.matmul(h1_ps[:, 0:P], lhsT=w1_sb[:, 0:P],
                                 rhs=aggT[:], start=True, stop=True)
                nc.tensor.matmul(h1_ps[:, P:HID_DIM], lhsT=w1_sb[:, P:HID_DIM],
                                 rhs=aggT[:], start=True, stop=True)
                h1T = wpool.tile([P, HID_DIM], F32, tag="h1T")
                nc.scalar.activation(h1T[:], h1_ps[:],
                                     mybir.ActivationFunctionType.Relu)
                m2_ps = m2_pp.tile([P, OUT_DIM], F32, tag="m2")
                nc.tensor.matmul(m2_ps[:], lhsT=h1T[:, 0:P],
                                 rhs=w2_sb[:, 0:P], start=True, stop=False)
                nc.tensor.matmul(m2_ps[:], lhsT=h1T[:, P:HID_DIM],
                                 rhs=w2_sb[:, P:HID_DIM], start=False,
                                 stop=True)
                m2_sb = wpool.tile([P, OUT_DIM], BF16, tag="m2sb")
                nc.vector.tensor_scalar_mul(m2_sb[:], m2_ps[:],
                                            di2_sb[:, t:t + 1])
                nc.sync.dma_start(t_m2[t * P:t * P + v, :], m2_sb[:v, :])

            _emit_aggregation(nc, meta, (mpool, spool, agg_pp), t_tbl,
                              idx_sb, dl_sb, iota_sb, IN_DIM, n_nodes, l1_tail)


def _build_layer2(nc, meta, n_tiles, n_valid, n_nodes, bout_val):
    """NEFF C: gather M2, aggregate, y = dinv*(relu(aggT)^T Wout)+bout."""
    NC_ = meta["n_chunks"]
    t_tbl = nc.dram_tensor("m2full", [n_nodes, OUT_DIM], BF16,
                           kind="ExternalInput")
    t_idx = nc.dram_tensor("idx", [P, NC_ * 8], mybir.dt.int16,
                           kind="ExternalInput")
    t_dl = nc.dram_tensor("dstloc", [P, NC_], F32, kind="ExternalInput")
    t_iota = nc.dram_tensor("iota", [P, P], BF16, kind="ExternalInput")
    t_wo = nc.dram_tensor("wout", [OUT_DIM, 1], F32, kind="ExternalInput")
    t_di = nc.dram_tensor("dinv_pp", [P, n_tiles], F32, kind="ExternalInput")
    t_y = nc.dram_tensor("y", [n_valid, 1], F32, kind="ExternalOutput")
    with tile.TileContext(nc) as tc:
        with (
            tc.tile_pool(name="const", bufs=1) as cpool,
            tc.tile_pool(name="msg", bufs=4) as mpool,
            tc.tile_pool(name="s", bufs=4) as spool,
            tc.tile_pool(name="work", bufs=3) as wpool,
            tc.tile_pool(name="agg_ps", bufs=4, space="PSUM") as agg_pp,
            tc.tile_pool(name="y_ps", bufs=2, space="PSUM") as y_pp,
        ):
            nc.gpsimd.load_library(library_config.mlp)
            idx_sb = cpool.tile([P, NC_ * 8], mybir.dt.int16)
            dl_sb = cpool.tile([P, NC_], F32)
            iota_sb = cpool.tile([P, P], BF16)
            wo_sb = cpool.tile([OUT_DIM, 1], F32)
            di_sb = cpool.tile([P, n_tiles], F32)
            nc.sync.dma_start(idx_sb[:], t_idx[:])
            nc.sync.dma_start(dl_sb[:], t_dl[:])
            nc.sync.dma_start(iota_sb[:], t_iota[:])
            nc.sync.dma_start(wo_sb[:], t_wo[:])
            nc.sync.dma_start(di_sb[:], t_di[:])

            def l2_tail(t, agg_ps):
                v = min(P, n_valid - t * P)
                o2T = wpool.tile([P, P], F32, tag="o2T")
                nc.scalar.activation(o2T[:], agg_ps[:],
                                     mybir.ActivationFunctionType.Relu)
                y_ps = y_pp.tile([P, 1], F32, tag="y")
                nc.tensor.matmul(y_ps[:], lhsT=o2T[:], rhs=wo_sb[:],
                                 start=True, stop=True)
                y_sb = wpool.tile([P, 1], F32, tag="ysb")
                nc.vector.tensor_scalar(y_sb[:], y_ps[:], di_sb[:, t:t + 1],
                                        float(bout_val),
                                        op0=mybir.AluOpType.mult,
                                        op1=mybir.AluOpType.add)
                nc.sync.dma_start(t_y[t * P:t * P + v, :], y_sb[:v, :])

            _emit_aggregation(nc, meta, (mpool, spool, agg_pp), t_tbl,
                              idx_sb, dl_sb, iota_sb, OUT_DIM, n_nodes,
                              l2_tail)


# --------------------------------------------------------------------------
# launch helpers
# --------------------------------------------------------------------------

def _run(nc, in_maps, sim_mode, out_names, trace=False):
    if sim_mode:
        from concourse import bass_interp
        sim = bass_interp.MultiCoreSim(nc, N_CORES)
        for c in range(N_CORES):
            for k, v in in_maps[c].items():
                sim.cores[c].tensor(k)[:] = v
        sim.simulate(check_with_hw=False)
        return [{o: np.array(sim.cores[c].mem_tensor(o)) for o in out_names}
                for c in range(N_CORES)], None
    nc.compile()
    res = run_bass_kernel_spmd(nc, in_maps, list(range(N_CORES)), trace=trace)
    return res.results, res.exec_time_ns


def kernel(x, edge_index, W1, b1, W2, b2, Wout, bout):
    _register_ntff_hook()
    x = np.asarray(x, np.float32)
    edge_index = np.asarray(edge_index)
    W1 = np.asarray(W1, np.float32)
    W2 = np.asarray(W2, np.float32)
    Wout = np.asarray(Wout, np.float32)
    bout = np.asarray(bout, np.float32)
    assert np.all(np.asarray(b1) == 0) and np.all(np.asarray(b2) == 0), \
        "kernel assumes b1=b2=0 (as produced by setup_inputs)"

    n_nodes = x.shape[0]
    shard = n_nodes // N_CORES
    n_tiles = -(-shard // P)
    n_buckets = -(-n_nodes // BUCKET)
    sim_mode = bool(os.environ.get("GCN_SIM"))
    trace = bool(os.environ.get("GCN_TRACE"))

    # ---- graph preprocessing ----
    src = edge_index[0].astype(np.int64)
    dst = edge_index[1].astype(np.int64)
    loop = np.arange(n_nodes, dtype=np.int64)
    src_all = np.concatenate([src, loop])
    dst_all = np.concatenate([dst, loop])
    deg = np.bincount(dst_all, minlength=n_nodes).astype(np.float32)
    dinv = np.where(deg > 0, 1.0 / np.sqrt(deg), 0.0).astype(np.float32)

    core_meta = []
    for c in range(N_CORES):
        sel = (dst_all >= c * shard) & (dst_all < (c + 1) * shard)
        core_meta.append(_preprocess_core(
            src_all[sel], dst_all[sel] - c * shard, n_tiles, n_buckets))
    meta = _merge_meta(core_meta, n_tiles, n_buckets)

    iota = np.broadcast_to(
        np.arange(P, dtype=np.float32), (P, P)).astype(ml_dtypes.bfloat16)
    w2p = np.concatenate([W2[:P, :], W2[P:, :]], axis=1)

    dinv_pp, dinv2_pp, xs_list = [], [], []
    for c in range(N_CORES):
        lo = c * shard
        xs = np.zeros((n_tiles * P, IN_DIM), np.float32)
        xs[:shard] = x[lo:lo + shard]
        dv = np.ones(n_tiles * P, np.float32)
        dv[:shard] = dinv[lo:lo + shard]
        dpp = dv.reshape(n_tiles, P).T.copy()
        xs_list.append(xs)
        dinv_pp.append(dpp)
        dinv2_pp.append(dpp * dpp)

    exec_times = []

    # ---- NEFF A: prescale ----
    nca = _new_nc(sim_mode)
    _build_prescale(nca, n_tiles, shard)
    res, t = _run(nca, [{"xs": xs_list[c], "dinv_pp": dinv_pp[c]}
                        for c in range(N_CORES)], sim_mode, ["m1s"], trace)
    exec_times.append(t)
    m1full = np.concatenate([res[c]["m1s"] for c in range(N_CORES)], axis=0)

    # ---- NEFF B: layer 1 ----
    ncb = _new_nc(sim_mode)
    _build_layer1(ncb, meta, n_tiles, shard, n_nodes)
    res, t = _run(ncb, [{
        "m1full": m1full, "idx": meta["core_idx"][c],
        "dstloc": meta["core_dstloc"][c], "iota": np.asarray(iota),
        "w1": W1, "w2p": w2p, "dinv2_pp": dinv2_pp[c],
    } for c in range(N_CORES)], sim_mode, ["m2s"], trace)
    exec_times.append(t)
    m2full = np.concatenate([res[c]["m2s"] for c in range(N_CORES)], axis=0)

    # ---- NEFF C: layer 2 ----
    ncc = _new_nc(sim_mode)
    _build_layer2(ncc, meta, n_tiles, shard, n_nodes,
                  float(bout.reshape(-1)[0]))
    res, t = _run(ncc, [{
        "m2full": m2full, "idx": meta["core_idx"][c],
        "dstloc": meta["core_dstloc"][c], "iota": np.asarray(iota),
        "wout": Wout, "dinv_pp": dinv_pp[c],
    } for c in range(N_CORES)], sim_mode, ["y"], trace)
    exec_times.append(t)

    kernel.exec_times_ns = exec_times
    kernel.last_exec_time_ns = (sum(exec_times) if all(
        t is not None for t in exec_times) else None)
    y = np.concatenate([res[c]["y"] for c in range(N_CORES)], axis=0)
    return y.astype(np.float32)


# revision 18
# speedup vs baseline: 2.8410x; 2.8410x over previous
"""GCN (2-layer + linear head) Trainium2 kernel, 8-core SPMD, 3 NEFF launches.

Math (reference-equivalent, using linearity of segment_sum and b1=b2=0):
    A = D^-1/2 (Adj+I) D^-1/2
    y = relu(A relu(A x W1) W2) Wout + bout
computed as
    M1 = diag(dinv) x                        (bf16 gather table)   [NEFF A]
    agg1T_raw[:, d] = sum_{e: dst=d} M1[src_e]                     [NEFF B]
    h1T = relu(W1^T agg1T_raw)      (dst dinv commutes thru relu, b1=0)
    M2 = diag(dinv^2) (h1 W2)       (bf16 gather table)
    agg2T_raw[:, d] = sum_{e: dst=d} M2[src_e]                     [NEFF C]
    y = diag(dinv) (relu(agg2T_raw)^T Wout) + bout

Between launches the host only concatenates the 8 per-core shard outputs
into the full gather table (no arithmetic on host beyond graph indexing).

Per core: 12500 dst nodes, ~212k edges (incl self loops), edges sorted by
(superblock of 4 dst tiles, src bucket of 25000, dst tile). Each 128-edge
chunk is single (bucket, tile): dma_gather (int16 idx relative to bucket
base) fetches the 128 source rows (partition=edge), a one-hot S matrix
(iota is_equal dstloc, bf16) scatters them via PE matmul into the tile's
PSUM accumulator in transposed orientation out[feat, dst] — so the dense
per-tile matmul chain needs no transposes anywhere.
"""

import os
import sys

if "/opt/trn_rl_repo" not in sys.path:
    sys.path.insert(0, "/opt/trn_rl_repo")

import numpy as np
import ml_dtypes

import concourse.bacc as bacc
import concourse.mybir as mybir
import concourse.tile as tile
from concourse import library_config
from concourse.bass_utils import run_bass_kernel_spmd

P = 128
N_CORES = 8
IN_DIM = 128
HID_DIM = 256
OUT_DIM = 128
BUCKET = 25000                    # gather-window size; int16 idx < 32768
SBT = 4                           # dst tiles per superblock (psum residency)
MAXG = 8                          # chunks per dma_gather (1024 idx ring cap)
BF16 = mybir.dt.bfloat16
F32 = mybir.dt.float32


def _register_ntff_hook():
    """Make trace=True usable under axon when antenv.axon_hooks is absent."""
    try:
        import antenv.axon_hooks  # noqa: F401
        return
    except ImportError:
        pass
    import types
    import antenv
    mod = types.ModuleType("antenv.axon_hooks")
    _h = [None]
    mod.set_axon_ntff_profile_hook = lambda h: _h.__setitem__(0, h)
    mod.get_axon_ntff_profile_hook = lambda: _h[0]
    sys.modules["antenv.axon_hooks"] = mod
    antenv.axon_hooks = mod
    try:
        from trn_agent_boot.trn_boot import _ntff_profile_via_ctypes
        mod.set_axon_ntff_profile_hook(
            _ntff_profile_via_ctypes("/opt/axon/libaxon_pjrt.so"))
    except Exception:
        pass


# --------------------------------------------------------------------------
# host-side graph preprocessing (pure index space)
# --------------------------------------------------------------------------

def _preprocess_core(s, d, n_tiles, n_buckets):
    """Edges (global src s, core-local dst d) -> sorted arrays + counts."""
    tile_id = d >> 7
    bucket = s // BUCKET
    sb = tile_id // SBT
    key = (sb.astype(np.int64) * n_buckets + bucket) * n_tiles + tile_id
    order = np.argsort(key, kind="stable")
    s, d, tile_id, bucket = (a[order] for a in (s, d, tile_id, bucket))
    counts = np.zeros((n_tiles, n_buckets), np.int64)
    np.add.at(counts, (tile_id, bucket), 1)
    return dict(s=s, d=d, tile_id=tile_id, bucket=bucket, counts=counts)


def _merge_meta(core_meta, n_tiles, n_buckets):
    """Canonical chunk/matmul layout shared by all 8 cores (one NEFF).

    Per (superblock, bucket) run: tiles laid back-to-back at EDGE
    granularity using canonical per-(tile,bucket) counts = max over cores;
    the run is padded to a multiple of 128 only at its end. Chunks (128
    edges) may straddle tile boundaries: each (chunk, tile) overlap gets
    its own matmul with its own one-hot S column (pad rows = -1)."""
    counts = np.stack([m["counts"] for m in core_meta])   # [8, T, B]
    canon = counts.max(axis=0)                            # [T, B]

    n_sb = -(-n_tiles // SBT)
    # canonical segments: per (sb, b): list of (tile, seg_start_edge, cnt)
    chunk_bucket = []        # per chunk
    groups = []              # (bucket, c_lo, c_hi, s_lo, s_hi)
    matmuls = []             # (chunk, tile, scol)
    seg_info = {}            # (t, b) -> (global_edge_pos, run_id)
    nch = 0
    edge_pos = 0             # global canonical edge position (idx stream)
    for sbi in range(n_sb):
        tiles = list(range(sbi * SBT, min((sbi + 1) * SBT, n_tiles)))
        for b in range(n_buckets):
            run_len = int(sum(canon[t, b] for t in tiles))
            if run_len == 0:
                continue
            run_pad = -run_len % P
            nch_run = (run_len + run_pad) // P
            c0 = nch
            # segment starts within run
            off = 0
            segs = []
            for t in tiles:
                cnt = int(canon[t, b])
                if cnt:
                    segs.append((t, off, cnt))
                    seg_info[(t, b)] = edge_pos + off
                off += cnt
            # matmuls: chunk x overlapping tiles
            for j in range(nch_run):
                lo, hi = j * P, (j + 1) * P
                for (t, soff, cnt) in segs:
                    if soff < hi and soff + cnt > lo:
                        matmuls.append([c0 + j, t, 0])
            chunk_bucket += [b] * nch_run
            # gather groups of up to MAXG chunks
            k = 0
            while k < nch_run:
                k2 = min(k + MAXG, nch_run)
                groups.append((b, c0 + k, c0 + k2))
                k = k2
            nch += nch_run
            edge_pos += nch_run * P

    chunk_bucket = np.asarray(chunk_bucket)
    # assign S columns in matmul order (grouped per chunk => contiguous
    # column ranges per gather group) and first/last flags per tile
    first = np.zeros(len(matmuls), bool)
    last = np.zeros(len(matmuls), bool)
    seen = {}
    for i, mm in enumerate(matmuls):
        mm[2] = i
        t = mm[1]
        if t not in seen:
            first[i] = True
        seen[t] = i
    for t, i in seen.items():
        last[i] = True
    n_scols = len(matmuls)
    # per-group matmul index range (matmuls sorted by chunk)
    mm_chunk = np.asarray([m[0] for m in matmuls])
    grp_mm = []
    for (b, c_lo, c_hi) in groups:
        i0 = int(np.searchsorted(mm_chunk, c_lo))
        i1 = int(np.searchsorted(mm_chunk, c_hi))
        grp_mm.append((i0, i1))

    # per-core idx / dstloc placement
    core_idx, core_dstloc = [], []
    for m in core_meta:
        idx_flat = np.zeros(nch * P, np.int16)
        dl_cols = np.full((n_scols, P), -1.0, np.float32)
        s_, d_ = m["s"], m["d"]
        # edges per (t,b) are contiguous in the sorted arrays
        cnts = m["counts"]
        core_seg = {}
        pos = 0
        # walk (sb, b, t) in the same canonical order as the sort key
        for sbi in range(n_sb):
            tiles = list(range(sbi * SBT, min((sbi + 1) * SBT, n_tiles)))
            for b in range(n_buckets):
                for t in tiles:
                    cnt = int(cnts[t, b])
                    if cnt == 0:
                        continue
                    gpos = seg_info[(t, b)]
                    idx_flat[gpos:gpos + cnt] = \
                        (s_[pos:pos + cnt] - b * BUCKET).astype(np.int16)
                    core_seg[(t, b)] = pos
                    pos += cnt
        # dstloc per matmul column: rows where this core's edges of (t,b)
        # fall inside the matmul's chunk
        for i, (ch, t, scol) in enumerate(matmuls):
            b = int(chunk_bucket[ch])
            if (t, b) not in core_seg:
                continue
            gpos = seg_info[(t, b)]
            cntc = int(cnts[t, b])
            lo = max(gpos, ch * P)
            hi = min(gpos + cntc, (ch + 1) * P)
            if lo < hi:
                r0, r1 = lo - ch * P, hi - ch * P
                e0 = lo - gpos
                p0 = core_seg[(t, b)] + e0
                dl_cols[scol, r0:r1] = \
                    (d_[p0:p0 + (hi - lo)] - t * P).astype(np.float32)
        idx16 = idx_flat.reshape(-1, 16).T.copy()
        core_idx.append(np.tile(idx16, (8, 1)))
        core_dstloc.append(dl_cols.T.copy())

    return dict(first=first, last=last, groups=groups, grp_mm=grp_mm,
                matmuls=[(m[0], m[1], m[2]) for m in matmuls],
                n_chunks=nch, n_scols=n_scols,
                core_idx=core_idx, core_dstloc=core_dstloc)


# --------------------------------------------------------------------------
# NEFF builders
# --------------------------------------------------------------------------

def _new_nc(sim_mode):
    return bacc.Bacc("TRN2", target_bir_lowering=False, debug=sim_mode,
                     num_devices=N_CORES, num_swdge_queues=4)


def _build_prescale(nc, n_tiles, n_valid):
    """NEFF A: m1s = bf16(diag(dinv) x_shard), transposed-tile layout.

    xs[p, t*128+j] = x[t*128+p, j]; m1s same layout, bf16."""
    t_x = nc.dram_tensor("xs", [P, n_tiles * IN_DIM], F32,
                         kind="ExternalInput")
    t_di = nc.dram_tensor("dinv_pp", [P, n_tiles], F32, kind="ExternalInput")
    t_m1 = nc.dram_tensor("m1s", [P, n_tiles * IN_DIM], BF16,
                          kind="ExternalOutput")
    with tile.TileContext(nc) as tc:
        with tc.tile_pool(name="w", bufs=1) as wpool:
            di_sb = wpool.tile([P, n_tiles], F32)
            nc.sync.dma_start(di_sb[:], t_di[:])
            x_sb = wpool.tile([P, n_tiles, IN_DIM], F32)
            nc.sync.dma_start(x_sb[:], t_x[:])
            m1_sb = wpool.tile([P, n_tiles, IN_DIM], BF16)
            nc.vector.tensor_tensor(
                out=m1_sb[:], in0=x_sb[:],
                in1=di_sb[:, :, None].to_broadcast([P, n_tiles, IN_DIM]),
                op=mybir.AluOpType.mult)
            nc.sync.dma_start(t_m1[:], m1_sb[:])


def _emit_aggregation(nc, meta, pools, table_ap, idx_sb, dl_sb, iota_sb,
                      feat_dim, n_nodes, tail_fn):
    """Gather + one-hot scatter matmuls; tail_fn(t, agg_psum) per dst tile."""
    mpool, spool, agg_pp = pools
    psums = {}
    matmuls = meta["matmuls"]
    for gi, ((b, c_lo, c_hi), (i0, i1)) in enumerate(
            zip(meta["groups"], meta["grp_mm"])):
        nch = c_hi - c_lo
        nidx = nch * P
        nmm = i1 - i0
        msg = mpool.tile([P, MAXG, feat_dim], BF16, tag="msg")
        nc.gpsimd.dma_gather(
            msg[:, :nch, :],
            table_ap[b * BUCKET:min((b + 1) * BUCKET, n_nodes), :],
            idx_sb[:, c_lo * 8:c_hi * 8],
            nidx, nidx, feat_dim, single_packet=False, queue_num=gi % 4)
        # one-hot S for all of the group's matmul columns in one DVE op:
        # S[p, k, j] = (dstloc[p, i0+k] == j)
        S = spool.tile([P, MAXG + SBT, P], BF16, tag="S")
        nc.vector.tensor_tensor(
            out=S[:, :nmm, :],
            in0=dl_sb[:, i0:i1].to_broadcast([P, nmm, P]),
            in1=iota_sb[:, None, :].to_broadcast([P, nmm, P]),
            op=mybir.AluOpType.is_equal)
        for i in range(i0, i1):
            ch, t, scol = matmuls[i]
            if meta["first"][i]:
                psums[t] = agg_pp.tile([P, P], F32, tag="agg",
                                       name=f"aggps_{t}")
            nc.tensor.matmul(psums[t][:], lhsT=msg[:, ch - c_lo, :],
                             rhs=S[:, i - i0, :],
                             start=bool(meta["first"][i]),
                             stop=bool(meta["last"][i]))
            if meta["last"][i]:
                tail_fn(t, psums.pop(t))


def _build_layer1(nc, meta, n_tiles, n_valid, n_nodes):
    """NEFF B: gather M1, aggregate, h1T = relu(W1^T aggT),
    m2s = bf16(diag(dinv^2) (h1 W2))."""
    NC_ = meta["n_chunks"]
    t_tbl = nc.dram_tensor("m1full", [n_nodes, IN_DIM], BF16,
                           kind="ExternalInput")
    t_idx = nc.dram_tensor("idx", [P, NC_ * 8], mybir.dt.int16,
                           kind="ExternalInput")
    t_dl = nc.dram_tensor("dstloc", [P, meta["n_scols"]], F32,
                          kind="ExternalInput")
    t_iota = nc.dram_tensor("iota", [P, P], BF16, kind="ExternalInput")
    t_w1 = nc.dram_tensor("w1", [IN_DIM, HID_DIM], F32, kind="ExternalInput")
    t_w2 = nc.dram_tensor("w2p", [P, HID_DIM], F32, kind="ExternalInput")
    t_di2 = nc.dram_tensor("dinv2_pp", [P, n_tiles], F32, kind="ExternalInput")
    t_m2 = nc.dram_tensor("m2s", [n_valid, OUT_DIM], BF16,
                          kind="ExternalOutput")
    with tile.TileContext(nc) as tc:
        with (
            tc.tile_pool(name="const", bufs=1) as cpool,
            tc.tile_pool(name="msg", bufs=8) as mpool,
            tc.tile_pool(name="s", bufs=4) as spool,
            tc.tile_pool(name="work", bufs=3) as wpool,
            tc.tile_pool(name="agg_ps", bufs=4, space="PSUM") as agg_pp,
            tc.tile_pool(name="h1_ps", bufs=2, space="PSUM") as h1_pp,
            tc.tile_pool(name="m2_ps", bufs=2, space="PSUM") as m2_pp,
        ):
            nc.gpsimd.load_library(library_config.mlp)
            idx_sb = cpool.tile([P, NC_ * 8], mybir.dt.int16)
            dl_sb = cpool.tile([P, meta["n_scols"]], F32)
            iota_sb = cpool.tile([P, P], BF16)
            w1_sb = cpool.tile([IN_DIM, HID_DIM], F32)
            w2_sb = cpool.tile([P, HID_DIM], F32)
            di2_sb = cpool.tile([P, n_tiles], F32)
            nc.sync.dma_start(idx_sb[:], t_idx[:])
            nc.sync.dma_start(dl_sb[:], t_dl[:])
            nc.sync.dma_start(iota_sb[:], t_iota[:])
            nc.sync.dma_start(w1_sb[:], t_w1[:])
            nc.sync.dma_start(w2_sb[:], t_w2[:])
            nc.sync.dma_start(di2_sb[:], t_di2[:])

            def l1_tail(t, agg_ps):
                v = min(P, n_valid - t * P)
                aggT = wpool.tile([P, P], F32, tag="aggT")
                nc.vector.tensor_copy(aggT[:], agg_ps[:])
                h1_ps = h1_pp.tile([P, HID_DIM], F32, tag="h1")
                nc.tensor.matmul(h1_ps[:, 0:P], lhsT=w1_sb[:, 0:P],
                                 rhs=aggT[:], start=True, stop=True)
                nc.tensor.matmul(h1_ps[:, P:HID_DIM], lhsT=w1_sb[:, P:HID_DIM],
                                 rhs=aggT[:], start=True, stop=True)
                h1T = wpool.tile([P, HID_DIM], F32, tag="h1T")
                nc.scalar.activation(h1T[:], h1_ps[:],
                                     mybir.ActivationFunctionType.Relu)
                m2_ps = m2_pp.tile([P, OUT_DIM], F32, tag="m2")
                nc.tensor.matmul(m2_ps[:], lhsT=h1T[:, 0:P],
                                 rhs=w2_sb[:, 0:P], start=True, stop=False)
                nc.tensor.matmul(m2_ps[:], lhsT=h1T[:, P:HID_DIM],
                                 rhs=w2_sb[:, P:HID_DIM], start=False,
                                 stop=True)
                m2_sb = wpool.tile([P, OUT_DIM], BF16, tag="m2sb")
                nc.vector.tensor_scalar_mul(m2_sb[:], m2_ps[:],
                                            di2_sb[:, t:t + 1])
                nc.sync.dma_start(t_m2[t * P:t * P + v, :], m2_sb[:v, :])

            _emit_aggregation(nc, meta, (mpool, spool, agg_pp), t_tbl,
                              idx_sb, dl_sb, iota_sb, IN_DIM, n_nodes, l1_tail)


def _build_layer2(nc, meta, n_tiles, n_valid, n_nodes, bout_val):
    """NEFF C: gather M2, aggregate, y = dinv*(relu(aggT)^T Wout)+bout."""
    NC_ = meta["n_chunks"]
    t_tbl = nc.dram_tensor("m2full", [n_nodes, OUT_DIM], BF16,
                           kind="ExternalInput")
    t_idx = nc.dram_tensor("idx", [P, NC_ * 8], mybir.dt.int16,
                           kind="ExternalInput")
    t_dl = nc.dram_tensor("dstloc", [P, meta["n_scols"]], F32,
                          kind="ExternalInput")
    t_iota = nc.dram_tensor("iota", [P, P], BF16, kind="ExternalInput")
    t_wo = nc.dram_tensor("wout", [OUT_DIM, 1], F32, kind="ExternalInput")
    t_di = nc.dram_tensor("dinv_pp", [P, n_tiles], F32, kind="ExternalInput")
    t_y = nc.dram_tensor("y", [n_valid, 1], F32, kind="ExternalOutput")
    with tile.TileContext(nc) as tc:
        with (
            tc.tile_pool(name="const", bufs=1) as cpool,
            tc.tile_pool(name="msg", bufs=8) as mpool,
            tc.tile_pool(name="s", bufs=4) as spool,
            tc.tile_pool(name="work", bufs=3) as wpool,
            tc.tile_pool(name="agg_ps", bufs=4, space="PSUM") as agg_pp,
            tc.tile_pool(name="y_ps", bufs=2, space="PSUM") as y_pp,
        ):
            nc.gpsimd.load_library(library_config.mlp)
            idx_sb = cpool.tile([P, NC_ * 8], mybir.dt.int16)
            dl_sb = cpool.tile([P, meta["n_scols"]], F32)
            iota_sb = cpool.tile([P, P], BF16)
            wo_sb = cpool.tile([OUT_DIM, 1], F32)
            di_sb = cpool.tile([P, n_tiles], F32)
            nc.sync.dma_start(idx_sb[:], t_idx[:])
            nc.sync.dma_start(dl_sb[:], t_dl[:])
            nc.sync.dma_start(iota_sb[:], t_iota[:])
            nc.sync.dma_start(wo_sb[:], t_wo[:])
            nc.sync.dma_start(di_sb[:], t_di[:])

            def l2_tail(t, agg_ps):
                v = min(P, n_valid - t * P)
                o2T = wpool.tile([P, P], F32, tag="o2T")
                nc.scalar.activation(o2T[:], agg_ps[:],
                                     mybir.ActivationFunctionType.Relu)
                y_ps = y_pp.tile([P, 1], F32, tag="y")
                nc.tensor.matmul(y_ps[:], lhsT=o2T[:], rhs=wo_sb[:],
                                 start=True, stop=True)
                y_sb = wpool.tile([P, 1], F32, tag="ysb")
                nc.vector.tensor_scalar(y_sb[:], y_ps[:], di_sb[:, t:t + 1],
                                        float(bout_val),
                                        op0=mybir.AluOpType.mult,
                                        op1=mybir.AluOpType.add)
                nc.sync.dma_start(t_y[t * P:t * P + v, :], y_sb[:v, :])

            _emit_aggregation(nc, meta, (mpool, spool, agg_pp), t_tbl,
                              idx_sb, dl_sb, iota_sb, OUT_DIM, n_nodes,
                              l2_tail)


# --------------------------------------------------------------------------
# launch helpers
# --------------------------------------------------------------------------

def _run(nc, in_maps, sim_mode, out_names, trace=False):
    if sim_mode:
        from concourse import bass_interp
        sim = bass_interp.MultiCoreSim(nc, N_CORES)
        for c in range(N_CORES):
            for k, v in in_maps[c].items():
                sim.cores[c].tensor(k)[:] = v
        sim.simulate(check_with_hw=False)
        return [{o: np.array(sim.cores[c].mem_tensor(o)) for o in out_names}
                for c in range(N_CORES)], None
    nc.compile()
    res = run_bass_kernel_spmd(nc, in_maps, list(range(N_CORES)), trace=trace)
    return res.results, res.exec_time_ns


def kernel(x, edge_index, W1, b1, W2, b2, Wout, bout):
    _register_ntff_hook()
    x = np.asarray(x, np.float32)
    edge_index = np.asarray(edge_index)
    W1 = np.asarray(W1, np.float32)
    W2 = np.asarray(W2, np.float32)
    Wout = np.asarray(Wout, np.float32)
    bout = np.asarray(bout, np.float32)
    assert np.all(np.asarray(b1) == 0) and np.all(np.asarray(b2) == 0), \
        "kernel assumes b1=b2=0 (as produced by setup_inputs)"

    n_nodes = x.shape[0]
    shard = n_nodes // N_CORES
    n_tiles = -(-shard // P)
    n_buckets = -(-n_nodes // BUCKET)
    sim_mode = bool(os.environ.get("GCN_SIM"))
    trace = bool(os.environ.get("GCN_TRACE"))

    # ---- graph preprocessing ----
    src = edge_index[0].astype(np.int64)
    dst = edge_index[1].astype(np.int64)
    loop = np.arange(n_nodes, dtype=np.int64)
    src_all = np.concatenate([src, loop])
    dst_all = np.concatenate([dst, loop])
    deg = np.bincount(dst_all, minlength=n_nodes).astype(np.float32)
    dinv = np.where(deg > 0, 1.0 / np.sqrt(deg), 0.0).astype(np.float32)

    # Degree-balanced node -> (core, tile) permutation: equalizes per-tile
    # in-degree so the canonical (max-over-cores) chunk padding is minimal.
    perm_pos = _balance_permutation(deg.astype(np.int64), n_nodes, shard,
                                    n_tiles)
    inv_perm = np.empty(n_nodes, np.int64)
    inv_perm[perm_pos] = np.arange(n_nodes)
    src_all = perm_pos[src_all]
    dst_all = perm_pos[dst_all]
    dinv_p = dinv[inv_perm]
    x_p = x[inv_perm]

    core_meta = []
    for c in range(N_CORES):
        sel = (dst_all >= c * shard) & (dst_all < (c + 1) * shard)
        core_meta.append(_preprocess_core(
            src_all[sel], dst_all[sel] - c * shard, n_tiles, n_buckets))
    meta = _merge_meta(core_meta, n_tiles, n_buckets)

    iota = np.broadcast_to(
        np.arange(P, dtype=np.float32), (P, P)).astype(ml_dtypes.bfloat16)
    w2p = np.concatenate([W2[:P, :], W2[P:, :]], axis=1)

    dinv_pp, dinv2_pp, xs_list = [], [], []
    for c in range(N_CORES):
        lo = c * shard
        xs = np.zeros((n_tiles * P, IN_DIM), np.float32)
        xs[:shard] = x_p[lo:lo + shard]
        # transposed-tile layout for NEFF A: xs_dev[p, t*IN+j] = xs[t*128+p, j]
        xs_dev = xs.reshape(n_tiles, P, IN_DIM).transpose(1, 0, 2) \
            .reshape(P, n_tiles * IN_DIM).copy()
        dv = np.ones(n_tiles * P, np.float32)
        dv[:shard] = dinv_p[lo:lo + shard]
        dpp = dv.reshape(n_tiles, P).T.copy()
        xs_list.append(xs_dev)
        dinv_pp.append(dpp)
        dinv2_pp.append(dpp * dpp)

    exec_times = []

    # ---- NEFF A: prescale ----
    nca = _new_nc(sim_mode)
    _build_prescale(nca, n_tiles, shard)
    res, t = _run(nca, [{"xs": xs_list[c], "dinv_pp": dinv_pp[c]}
                        for c in range(N_CORES)], sim_mode, ["m1s"], trace)
    exec_times.append(t)
    m1full = np.concatenate([
        res[c]["m1s"].reshape(P, n_tiles, IN_DIM).transpose(1, 0, 2)
        .reshape(n_tiles * P, IN_DIM)[:shard]
        for c in range(N_CORES)], axis=0)

    # ---- NEFF B: layer 1 ----
    ncb = _new_nc(sim_mode)
    _build_layer1(ncb, meta, n_tiles, shard, n_nodes)
    res, t = _run(ncb, [{
        "m1full": m1full, "idx": meta["core_idx"][c],
        "dstloc": meta["core_dstloc"][c], "iota": np.asarray(iota),
        "w1": W1, "w2p": w2p, "dinv2_pp": dinv2_pp[c],
    } for c in range(N_CORES)], sim_mode, ["m2s"], trace)
    exec_times.append(t)
    m2full = np.concatenate([res[c]["m2s"] for c in range(N_CORES)], axis=0)

    # ---- NEFF C: layer 2 ----
    ncc = _new_nc(sim_mode)
    _build_layer2(ncc, meta, n_tiles, shard, n_nodes,
                  float(bout.reshape(-1)[0]))
    res, t = _run(ncc, [{
        "m2full": m2full, "idx": meta["core_idx"][c],
        "dstloc": meta["core_dstloc"][c], "iota": np.asarray(iota),
        "wout": Wout, "dinv_pp": dinv_pp[c],
    } for c in range(N_CORES)], sim_mode, ["y"], trace)
    exec_times.append(t)

    kernel.exec_times_ns = exec_times
    kernel.last_exec_time_ns = (sum(exec_times) if all(
        t is not None for t in exec_times) else None)
    y_p = np.concatenate([res[c]["y"] for c in range(N_CORES)], axis=0)
    return y_p[perm_pos].astype(np.float32)


def _balance_permutation(deg, n_nodes, shard, n_tiles):
    """Assign nodes to global tiles (128 slots each; each core's last tile
    holds shard - 128*(n_tiles-1)) balancing per-tile total degree.
    Returns perm_pos[orig_node] = permuted position."""
    import heapq
    last_cap = shard - P * (n_tiles - 1)
    caps = []
    for c in range(N_CORES):
        caps += [P] * (n_tiles - 1) + [last_cap]
    ntile_g = len(caps)
    order = np.argsort(-deg, kind="stable")
    heap = [(0, t) for t in range(ntile_g)]
    heapq.heapify(heap)
    fill = np.zeros(ntile_g, np.int64)
    base = np.zeros(ntile_g, np.int64)
    pos = 0
    for c in range(N_CORES):
        for t in range(n_tiles):
            base[c * n_tiles + t] = c * shard + t * P
    perm_pos = np.empty(n_nodes, np.int64)
    for n in order:
        while True:
            load, t = heapq.heappop(heap)
            if fill[t] < caps[t]:
                break
        perm_pos[n] = base[t] + fill[t]
        fill[t] += 1
        load += int(deg[n])
        if fill[t] < caps[t]:
            heapq.heappush(heap, (load, t))
    return perm_pos
